# revision 21
# baseline (speedup 1.0000x reference)
"""Bilinear RoI pooling (grid_sample style) on 8 Trainium2 NeuronCores.

Strategy (data-parallel over boxes; all coordinate math host-side):
  - The affine grid is axis-aligned, so sampling is separable: each output
    point (b, oy, ox) is a 2x2 bilinear blend. Per box only ~6 distinct
    feature rows are touched by all 7 oy grid rows (box heights are 8-64 px
    on a stride-8 map), so the kernel gathers one descriptor per
    (box, x-window, distinct-y-row) instead of one per sample point: a 2C
    fp16 window [f(y,x), f(y,x+1)] from a row-major [HW+1, C] table
    (elem_step=C, elem_size=2C). The x-windows are a minimal cover of the
    cells the box's 7 ox columns touch (narrow boxes share windows).
  - Descriptors are packed into tiles of <=128 descriptors covering <=18
    output columns (col = (b, ox); M = 7*18 = 126 output points <= 128).
  - Per tile, lhsT[k, (cl, oy)] factorizes as cmq[q][k, cl] * wyw[k, oy]
    (x-corner weight folded into the column mask), densified on-device by
    one DVE broadcast-multiply per (chunk, q). Two accumulating fp16
    matmuls per tile (q = left/right C half of the gathered window) produce
    PSUM [126 pts, C]; PSUM -> SBUF (alternating DVE/ACT) -> DRAM fp16.
  - Gathers use prepare_only SWDGE + trigger_dma so Pool-engine descriptor
    prep overlaps the DMA transfers. Each call's trailing pad descriptors
    are -1 (skipped by the DGE); the per-call valid count is read from an
    input tensor into a Pool register (per-core counts differ).
  - Host transposes the per-tile point blocks back to [B, C, 7, 7].
"""
import sys
import numpy as np

sys.path.insert(0, "/opt/trn_rl_repo")

OH = OW = 7
C, H, W = 512, 64, 256
HW = H * W
B_TOTAL = 2048
N_CORES = 8
B_LOCAL = B_TOTAL // N_CORES
MAX_COLS = 18            # columns per tile -> M = 126
MAX_DESC = 128           # descriptors per tile (K partitions)
M = MAX_COLS * OH        # 126
CALL_TILES = 8           # tiles per dma_gather call
GPOOL_BUFS = 7           # gather pool depth (pipeline)
PREPARE_ONLY = False      # SWDGE prepare_only + trigger_dma pipelining
NEG_PADS = True          # trailing -1 pads + per-call register counts


def _build(nc, tc, NT):
    from contextlib import ExitStack
    import concourse.mybir as mybir
    from concourse import bass

    f32 = mybir.dt.float32
    f16 = mybir.dt.float16
    i16 = mybir.dt.int16
    i32 = mybir.dt.int32
    A = mybir.AluOpType
    Pool = mybir.EngineType.Pool

    NCALLS = (NT + CALL_TILES - 1) // CALL_TILES

    feats_t = nc.dram_tensor("feats_t", [HW + 1, C], f16, kind="ExternalInput")
    idxw_d = nc.dram_tensor("idxw", [128, NT * 8], i16, kind="ExternalInput")
    cmq_d = nc.dram_tensor("cmq", [128, NT * 2 * MAX_COLS], f16,
                           kind="ExternalInput")
    wyw_d = nc.dram_tensor("wyw", [128, NT * OH], f16, kind="ExternalInput")
    cnt_d = nc.dram_tensor("cnt", [1, NCALLS], i32, kind="ExternalInput")
    out_d = nc.dram_tensor("out3", [128, NT * C], f16, kind="ExternalOutput")

    es = ExitStack()
    idx0_s = es.enter_context(
        nc.sbuf_tensor("idx0_s", [128, CALL_TILES * 8], i16))
    idx_s = es.enter_context(nc.sbuf_tensor("idx_s", [128, NT * 8], i16))
    cmq_s = es.enter_context(
        nc.sbuf_tensor("cmq_s", [128, NT, 2, MAX_COLS], f16))
    wyw_s = es.enter_context(nc.sbuf_tensor("wyw_s", [128, NT, OH], f16))
    cnt_s = es.enter_context(nc.sbuf_tensor("cnt_s", [1, NCALLS], i32))

    # gather source: row pitch C, window 2C (rows i and i+1 = x, x+1)
    src_ap = bass.AP(feats_t, 0, [[C, HW], [1, 2 * C]])

    ncmq = NT * 2 * MAX_COLS
    nwyw = NT * OH

    dma_sems = [nc.alloc_semaphore(f"swdge_dma{q}") for q in range(4)]

    with tc.tile_pool(name="gpool", bufs=GPOOL_BUFS) as gpool, \
         tc.tile_pool(name="wpool", bufs=3) as wpool, \
         tc.tile_pool(name="spool", bufs=3) as spool, \
         tc.tile_pool(name="psum", bufs=8, space="PSUM") as psum_pool:
        # split the idx/cnt loads so the first gather can start as soon as
        # its own slice lands, instead of after all weight tensors
        c0 = CALL_TILES * 8
        nc.sync.dma_start(out=idx0_s[:, :], in_=idxw_d[:, :c0])
        nc.sync.dma_start(out=cnt_s[:, :], in_=cnt_d[:, :])
        nc.sync.dma_start(out=idx_s[:, :], in_=idxw_d[:, :])
        nc.sync.dma_start(
            out=bass.AP(cmq_s, 0, [[ncmq, 128], [1, ncmq]]), in_=cmq_d[:, :])
        nc.sync.dma_start(
            out=bass.AP(wyw_s, 0, [[nwyw, 128], [1, nwyw]]), in_=wyw_d[:, :])

        t0 = 0
        for ci in range(NCALLS):
            k = min(CALL_TILES, NT - t0)
            q = ci % 4
            Gt = gpool.tile([128, CALL_TILES, 2 * C], f16, name="Gt")
            if NEG_PADS:
                reg = nc.alloc_registers(f"cnt{ci}", engines=[Pool])
                nc.regs_load(reg, cnt_s[0:1, ci:ci + 1])
                cnt_val = nc.snap(reg, donate=True, min_val=1,
                                  max_val=k * MAX_DESC)
            else:
                cnt_val = k * 128
            iap = (idx0_s[:, :k * 8] if ci == 0
                   else idx_s[:, t0 * 8: (t0 + k) * 8])
            nc.gpsimd.dma_gather(
                out_ap=Gt[:, :k, :], in_ap=src_ap,
                idxs_ap=iap,
                num_idxs=k * 128, num_idxs_reg=cnt_val,
                elem_size=2 * C, elem_step=C, queue_num=q)
            # densify lhsT: wden[p, ti, q, cl*7+oy] = cmq[p,t,q,cl]*wyw[p,t,oy]
            wden = wpool.tile([128, CALL_TILES, 2, M], f16, name="wden")
            for xq in range(2):
                nc.vector.tensor_tensor(
                    out=wden[:, :k, xq, :],
                    in0=bass.AP(cmq_s, (t0 * 2 + xq) * MAX_COLS,
                                [[NT * 2 * MAX_COLS, 128],
                                 [2 * MAX_COLS, k], [1, MAX_COLS], [0, OH]]),
                    in1=bass.AP(wyw_s, t0 * OH,
                                [[NT * OH, 128], [OH, k], [0, MAX_COLS],
                                 [1, OH]]),
                    op=A.mult)
            stage = spool.tile([128, CALL_TILES, C], f16, name="stage")
            for ti in range(k):
                t = t0 + ti
                ps = psum_pool.tile([128, C], f32, name="ps")
                for xq in range(2):
                    nc.tensor.matmul(
                        out=ps[:M, :],
                        lhsT=wden[:, ti, xq, :],
                        rhs=Gt[:, ti, xq * C:(xq + 1) * C],
                        start=(xq == 0), stop=(xq == 1))
                dst = stage[:M, ti, :]
                if t % 2 == 0:
                    nc.vector.tensor_copy(out=dst, in_=ps[:M, :])
                else:
                    nc.scalar.activation(
                        out=dst, in_=ps[:M, :],
                        func=mybir.ActivationFunctionType.Copy)
            nc.sync.dma_start(
                out=bass.AP(out_d, t0 * C, [[NT * C, M], [1, k * C]]),
                in_=stage[:M, :k, :])
            t0 += k


def _box_geometry(boxes, Him, Wim):
    xc, yc, bw, bh = [boxes[:, i].astype(np.float64) for i in range(4)]
    gl = np.linspace(-1.0, 1.0, 7)
    gx = gl[None, :] * ((bw - 1) / (Wim - 1))[:, None] \
        + ((2 * xc - Wim - 1) / (Wim - 1))[:, None]
    gy = gl[None, :] * ((bh - 1) / (Him - 1))[:, None] \
        + ((2 * yc - Him - 1) / (Him - 1))[:, None]
    ix = np.clip((gx + 1) * 0.5 * (W - 1), 0.0, W - 1.0)
    iy = np.clip((gy + 1) * 0.5 * (H - 1), 0.0, H - 1.0)
    x0 = np.floor(ix).astype(np.int64)
    y0 = np.floor(iy).astype(np.int64)
    wx = ix - x0
    wy = iy - y0
    y1 = np.minimum(y0 + 1, H - 1)
    return x0, wx, y0, y1, wy


def _cover(a_vals, wx_vals, oxs):
    """Minimal 2-cell window cover of the cells needed by columns `oxs`.

    Returns list of (xw, [(col_local_idx, q, weight), ...]) where q selects
    the left/right C half of the gathered window."""
    need = {}   # cell -> list of (ci, weight)
    for ci, ox in enumerate(oxs):
        a = int(a_vals[ox])
        need.setdefault(a, []).append((ci, 1.0 - wx_vals[ox]))
        need.setdefault(a + 1, []).append((ci, wx_vals[ox]))
    cells = sorted(need)
    wins = []
    i = 0
    while i < len(cells):
        xw = cells[i]
        cov = [(xw, 0)]
        if i + 1 < len(cells) and cells[i + 1] == xw + 1:
            cov.append((xw + 1, 1))
            i += 2
        else:
            i += 1
        entries = []
        for cell, q in cov:
            for (ci, wgt) in need[cell]:
                entries.append((ci, q, wgt))
        wins.append((xw, entries))
    return wins


def _prep_core(boxes, Him, Wim):
    """Tile metadata for one core's boxes."""
    B = boxes.shape[0]
    x0, wx, y0, y1, wy = _box_geometry(boxes, Him, Wim)

    box_rows = []
    for b in range(B):
        acc = {}
        for oy in range(7):
            w0 = 1.0 - wy[b, oy]
            w1 = wy[b, oy]
            if w0 > 0:
                acc.setdefault(y0[b, oy], np.zeros(7))[oy] += w0
            if w1 > 0:
                acc.setdefault(y1[b, oy], np.zeros(7))[oy] += w1
        ys = sorted(acc.keys())
        box_rows.append((ys, np.stack([acc[y] for y in ys], 0)))

    # pack boxes into tiles, splitting a box's ox-range when it doesn't fit
    tiles = []        # list of list of (b, oxs, wins)
    cur, cc, cd = [], 0, 0
    for b in range(B):
        ny = len(box_rows[b][0])
        oxs = list(range(7))
        while oxs:
            # how many cols fit by the column cap?
            room_c = MAX_COLS - cc
            if room_c == 0:
                tiles.append(cur)
                cur, cc, cd = [], 0, 0
                room_c = MAX_COLS
            take = oxs[:room_c]
            wins = _cover(x0[b], wx[b], take)
            while len(take) > 1 and cd + ny * len(wins) > MAX_DESC:
                take = take[:-1]
                wins = _cover(x0[b], wx[b], take)
            if cd + ny * len(wins) > MAX_DESC:
                tiles.append(cur)
                cur, cc, cd = [], 0, 0
                continue
            cur.append((b, take, wins))
            cc += len(take)
            cd += ny * len(wins)
            oxs = oxs[len(take):]
    if cur:
        tiles.append(cur)

    NT = len(tiles)
    idx = np.zeros((NT, MAX_DESC), np.int32)
    nval = np.zeros(NT, np.int32)
    cmq = np.zeros((NT, 2, MAX_DESC, MAX_COLS), np.float16)
    wyw = np.zeros((NT, MAX_DESC, OH), np.float16)
    meta = []
    for t, parts in enumerate(tiles):
        k = 0
        cl = 0
        colmap = []
        for (b, take, wins) in parts:
            ys, wrow = box_rows[b]
            ny = len(ys)
            for (xw, entries) in wins:
                for j, y in enumerate(ys):
                    idx[t, k + j] = y * W + xw
                    wyw[t, k + j] = wrow[j].astype(np.float16)
                    for (ci, q, wgt) in entries:
                        cmq[t, q, k + j, cl + ci] = np.float16(wgt)
                k += ny
            for ox in take:
                colmap.append((b, ox))
            cl += len(take)
        nval[t] = k
        meta.append(colmap)
    return idx, nval, cmq, wyw, meta


def _pack_core(idx, nval, cmq, wyw, NT):
    """Pad to NT tiles, order tiles (emptiest last per call), build device
    tensors. Returns (idxw, cmq_dev, wyw_dev, counts, order)."""
    nt = idx.shape[0]
    NCALLS = (NT + CALL_TILES - 1) // CALL_TILES
    # order tiles: sort by valid count desc (pad tiles = emptiest, at the
    # back); each call takes CALL_TILES-1 from the front and its trailing
    # slot from the back, so the trailing (neg-padded) tile skips the most
    # gather bytes.
    assert NT - nt <= NCALLS, "more pad tiles than calls"
    ext = sorted(range(nt), key=lambda t: -nval[t]) + [-1] * (NT - nt)
    front, back = 0, NT - 1
    order = []
    for c in range(NCALLS):
        want = min(CALL_TILES, NT - c * CALL_TILES)
        for _ in range(want - 1):
            order.append(ext[front])
            front += 1
        order.append(ext[back])
        back -= 1
    assert len(order) == NT and front == back + 1
    real = [t for t in order if t >= 0]
    assert sorted(real) == list(range(nt))

    flat = np.full(NT * MAX_DESC, -1 if NEG_PADS else 0, np.int16)
    cmq_p = np.zeros((NT, 2, MAX_DESC, MAX_COLS), np.float16)
    wyw_p = np.zeros((NT, MAX_DESC, OH), np.float16)
    counts = np.zeros((1, NCALLS), np.int32)
    for pos, t in enumerate(order):
        call = pos // CALL_TILES
        # trailing tile of a call may use skipped -1 pads -- but the first
        # 3 calls (first use of each gather pool buffer) must write every
        # slot so later skipped slots only ever expose stale finite data
        last_in_call = ((pos % CALL_TILES == CALL_TILES - 1)
                        or pos == NT - 1) and call >= GPOOL_BUFS
        base = pos * MAX_DESC
        nv = 0
        if t >= 0:
            nv = int(nval[t])
            flat[base:base + nv] = idx[t, :nv].astype(np.int16)
            cmq_p[pos] = cmq[t]
            wyw_p[pos] = wyw[t]
        if not (NEG_PADS and last_in_call):
            # interior tiles: pads must be valid indices (transferred)
            flat[base + nv:base + MAX_DESC] = 0
            counts[0, call] += MAX_DESC
        else:
            counts[0, call] += nv
    if not NEG_PADS:
        for c in range(NCALLS):
            w = min(CALL_TILES, NT - c * CALL_TILES)
            counts[0, c] = w * MAX_DESC
    # any call whose tiles are all pads still needs >= 1 valid descriptor
    for c in range(NCALLS):
        if counts[0, c] == 0:
            flat[c * CALL_TILES * MAX_DESC] = 0
            counts[0, c] = 1

    idxw = np.zeros((16, NT * 8), np.int16)
    j = np.arange(NT * MAX_DESC)
    idxw[j % 16, j // 16] = flat
    idxw = np.tile(idxw, (8, 1))

    cmq_dev = np.ascontiguousarray(
        cmq_p.transpose(2, 0, 1, 3)).reshape(128, NT * 2 * MAX_COLS)
    wyw_dev = np.ascontiguousarray(
        wyw_p.transpose(1, 0, 2)).reshape(128, NT * OH)
    return idxw, cmq_dev, wyw_dev, counts, order


_CACHE = {}


def _get_compiled(NT):
    key = ("nc", NT)
    if key in _CACHE:
        return _CACHE[key]
    import concourse.bacc as bacc
    import concourse.tile as tile
    nc = bacc.Bacc("TRN2", target_bir_lowering=False, debug=False,
                   num_swdge_queues=4)
    with tile.TileContext(nc) as tc:
        _build(nc, tc, NT)
    nc.compile()
    _CACHE[key] = nc
    return nc


def _run(feats, boxes, Him, Wim, trace=False, tmpdir=None):
    from concourse.bass_utils import run_bass_kernel_spmd

    table = np.zeros((HW + 1, C), np.float16)
    table[:HW] = feats.transpose(1, 2, 0).reshape(HW, C).astype(np.float16)

    preps = []
    for i in range(N_CORES):
        preps.append(_prep_core(boxes[i * B_LOCAL:(i + 1) * B_LOCAL],
                                Him, Wim))
    NT = max(p[0].shape[0] for p in preps)
    nc = _get_compiled(NT)

    in_maps = []
    orders = []
    for i in range(N_CORES):
        idx, nval, cmq, wyw, _ = preps[i]
        idxw, cmq_dev, wyw_dev, counts, order = _pack_core(
            idx, nval, cmq, wyw, NT)
        orders.append(order)
        in_maps.append({"feats_t": table, "idxw": idxw, "cmq": cmq_dev,
                        "wyw": wyw_dev, "cnt": counts})
    res = run_bass_kernel_spmd(nc, in_maps, list(range(N_CORES)),
                               trace=trace, tmpdir=tmpdir)

    out = np.zeros((B_TOTAL, C, OH, OW), np.float32)
    for i in range(N_CORES):
        o = np.asarray(res.results[i]["out3"], np.float32)
        o = o.reshape(128, NT, C)
        meta = preps[i][4]
        for pos, t in enumerate(orders[i]):
            if t < 0:
                continue
            colmap = meta[t]
            blk = o[:len(colmap) * OH, pos, :]      # [ncols*7, C]
            for ci, (b, ox) in enumerate(colmap):
                out[i * B_LOCAL + b, :, :, ox] = blk[ci * 7:(ci + 1) * 7].T
    return out, res


def kernel(**inputs):
    feats = np.asarray(inputs["feats"], dtype=np.float32)
    boxes = np.asarray(inputs["boxes"], dtype=np.float32)
    Him = int(inputs["image_height"])
    Wim = int(inputs["image_width"])
    out, _ = _run(feats, boxes, Him, Wim, trace=False)
    return out


# revision 22
# speedup vs baseline: 1.0439x; 1.0439x over previous
"""Bilinear RoI pooling (grid_sample style) on 8 Trainium2 NeuronCores.

Strategy (data-parallel over boxes; all coordinate math host-side):
  - The affine grid is axis-aligned, so sampling is separable: each output
    point (b, oy, ox) is a 2x2 bilinear blend. Per box only ~6 distinct
    feature rows are touched by all 7 oy grid rows (box heights are 8-64 px
    on a stride-8 map), so the kernel gathers one descriptor per
    (box, x-window, distinct-y-row) instead of one per sample point: a 2C
    fp16 window [f(y,x), f(y,x+1)] from a row-major [HW+1, C] table
    (elem_step=C, elem_size=2C). The x-windows are a minimal cover of the
    cells the box's 7 ox columns touch (narrow boxes share windows).
  - Descriptors are packed into tiles of <=128 descriptors covering <=18
    output columns (col = (b, ox); M = 7*18 = 126 output points <= 128).
  - Per tile, lhsT[k, (cl, oy)] factorizes as cmq[q][k, cl] * wyw[k, oy]
    (x-corner weight folded into the column mask), densified on-device by
    one DVE broadcast-multiply per (chunk, q). Two accumulating fp16
    matmuls per tile (q = left/right C half of the gathered window) produce
    PSUM [126 pts, C]; PSUM -> SBUF (alternating DVE/ACT) -> DRAM fp16.
  - Gathers use prepare_only SWDGE + trigger_dma so Pool-engine descriptor
    prep overlaps the DMA transfers. Each call's trailing pad descriptors
    are -1 (skipped by the DGE); the per-call valid count is read from an
    input tensor into a Pool register (per-core counts differ).
  - Host transposes the per-tile point blocks back to [B, C, 7, 7].
"""
import sys
import numpy as np

sys.path.insert(0, "/opt/trn_rl_repo")

OH = OW = 7
C, H, W = 512, 64, 256
HW = H * W
B_TOTAL = 2048
N_CORES = 8
B_LOCAL = B_TOTAL // N_CORES
MAX_COLS = 18            # columns per tile -> M = 126
MAX_DESC = 128           # descriptors per tile (K partitions)
M = MAX_COLS * OH        # 126
CALL_TILES = 8           # tiles per dma_gather call
GPOOL_BUFS = 6           # gather pool depth (pipeline)
PREPARE_ONLY = False      # SWDGE prepare_only + trigger_dma pipelining
NEG_PADS = True          # trailing -1 pads + per-call register counts


def _build(nc, tc, NT):
    from contextlib import ExitStack
    import concourse.mybir as mybir
    from concourse import bass

    f32 = mybir.dt.float32
    f16 = mybir.dt.float16
    i16 = mybir.dt.int16
    i32 = mybir.dt.int32
    A = mybir.AluOpType
    Pool = mybir.EngineType.Pool

    NCALLS = (NT + CALL_TILES - 1) // CALL_TILES

    feats_t = nc.dram_tensor("feats_t", [HW + 1, C], f16, kind="ExternalInput")
    idxw_d = nc.dram_tensor("idxw", [128, NT * 8], i16, kind="ExternalInput")
    cmq_d = nc.dram_tensor("cmq", [128, NT * 2 * MAX_COLS], f16,
                           kind="ExternalInput")
    wyw_d = nc.dram_tensor("wyw", [128, NT * OH], f16, kind="ExternalInput")
    cnt_d = nc.dram_tensor("cnt", [1, NCALLS], i32, kind="ExternalInput")
    out_d = nc.dram_tensor("out3", [128, NT * C], f16, kind="ExternalOutput")

    es = ExitStack()
    idx0_s = es.enter_context(
        nc.sbuf_tensor("idx0_s", [128, CALL_TILES * 8], i16))
    idx_s = es.enter_context(nc.sbuf_tensor("idx_s", [128, NT * 8], i16))
    cmq_s = es.enter_context(
        nc.sbuf_tensor("cmq_s", [128, NT, 2, MAX_COLS], f16))
    wyw_s = es.enter_context(nc.sbuf_tensor("wyw_s", [128, NT, OH], f16))
    cnt_s = es.enter_context(nc.sbuf_tensor("cnt_s", [1, NCALLS], i32))

    # gather source: row pitch C, window 2C (rows i and i+1 = x, x+1)
    src_ap = bass.AP(feats_t, 0, [[C, HW], [1, 2 * C]])

    ncmq = NT * 2 * MAX_COLS
    nwyw = NT * OH

    dma_sems = [nc.alloc_semaphore(f"swdge_dma{q}") for q in range(4)]

    with tc.tile_pool(name="gpool", bufs=GPOOL_BUFS) as gpool, \
         tc.tile_pool(name="wpool", bufs=3) as wpool, \
         tc.tile_pool(name="spool", bufs=6) as spool, \
         tc.tile_pool(name="psum", bufs=8, space="PSUM") as psum_pool:
        # split the idx/cnt loads so the first gather can start as soon as
        # its own slice lands, instead of after all weight tensors
        c0 = CALL_TILES * 8
        nc.sync.dma_start(out=idx0_s[:, :], in_=idxw_d[:, :c0])
        nc.sync.dma_start(out=cnt_s[:, :], in_=cnt_d[:, :])
        nc.sync.dma_start(out=idx_s[:, c0:], in_=idxw_d[:, c0:])
        nc.sync.dma_start(
            out=bass.AP(cmq_s, 0, [[ncmq, 128], [1, ncmq]]), in_=cmq_d[:, :])
        nc.sync.dma_start(
            out=bass.AP(wyw_s, 0, [[nwyw, 128], [1, nwyw]]), in_=wyw_d[:, :])

        t0 = 0
        for ci in range(NCALLS):
            k = min(CALL_TILES, NT - t0)
            q = ci % 4
            Gt = gpool.tile([128, CALL_TILES, 2 * C], f16, name="Gt")
            if NEG_PADS:
                reg = nc.alloc_registers(f"cnt{ci}", engines=[Pool])
                nc.regs_load(reg, cnt_s[0:1, ci:ci + 1])
                cnt_val = nc.snap(reg, donate=True, min_val=1,
                                  max_val=k * MAX_DESC)
            else:
                cnt_val = k * 128
            iap = (idx0_s[:, :k * 8] if ci == 0
                   else idx_s[:, t0 * 8: (t0 + k) * 8])
            nc.gpsimd.dma_gather(
                out_ap=Gt[:, :k, :], in_ap=src_ap,
                idxs_ap=iap,
                num_idxs=k * 128, num_idxs_reg=cnt_val,
                elem_size=2 * C, elem_step=C, queue_num=q)
            # densify lhsT: wden[p, ti, q, cl*7+oy] = cmq[p,t,q,cl]*wyw[p,t,oy]
            wden = wpool.tile([128, CALL_TILES, 2, M], f16, name="wden")
            for xq in range(2):
                nc.vector.tensor_tensor(
                    out=wden[:, :k, xq, :],
                    in0=bass.AP(cmq_s, (t0 * 2 + xq) * MAX_COLS,
                                [[NT * 2 * MAX_COLS, 128],
                                 [2 * MAX_COLS, k], [1, MAX_COLS], [0, OH]]),
                    in1=bass.AP(wyw_s, t0 * OH,
                                [[NT * OH, 128], [OH, k], [0, MAX_COLS],
                                 [1, OH]]),
                    op=A.mult)
            stage = spool.tile([128, CALL_TILES, C], f16, name="stage")
            for ti in range(k):
                t = t0 + ti
                ps = psum_pool.tile([128, C], f32, name="ps")
                for xq in range(2):
                    nc.tensor.matmul(
                        out=ps[:M, :],
                        lhsT=wden[:, ti, xq, :],
                        rhs=Gt[:, ti, xq * C:(xq + 1) * C],
                        start=(xq == 0), stop=(xq == 1))
                dst = stage[:M, ti, :]
                if t % 2 == 0:
                    nc.vector.tensor_copy(out=dst, in_=ps[:M, :])
                else:
                    nc.scalar.activation(
                        out=dst, in_=ps[:M, :],
                        func=mybir.ActivationFunctionType.Copy)
            nc.sync.dma_start(
                out=bass.AP(out_d, t0 * C, [[NT * C, M], [1, k * C]]),
                in_=stage[:M, :k, :])
            t0 += k


def _box_geometry(boxes, Him, Wim):
    xc, yc, bw, bh = [boxes[:, i].astype(np.float64) for i in range(4)]
    gl = np.linspace(-1.0, 1.0, 7)
    gx = gl[None, :] * ((bw - 1) / (Wim - 1))[:, None] \
        + ((2 * xc - Wim - 1) / (Wim - 1))[:, None]
    gy = gl[None, :] * ((bh - 1) / (Him - 1))[:, None] \
        + ((2 * yc - Him - 1) / (Him - 1))[:, None]
    ix = np.clip((gx + 1) * 0.5 * (W - 1), 0.0, W - 1.0)
    iy = np.clip((gy + 1) * 0.5 * (H - 1), 0.0, H - 1.0)
    x0 = np.floor(ix).astype(np.int64)
    y0 = np.floor(iy).astype(np.int64)
    wx = ix - x0
    wy = iy - y0
    y1 = np.minimum(y0 + 1, H - 1)
    return x0, wx, y0, y1, wy


def _cover(a_vals, wx_vals, oxs):
    """Minimal 2-cell window cover of the cells needed by columns `oxs`.

    Returns list of (xw, [(col_local_idx, q, weight), ...]) where q selects
    the left/right C half of the gathered window."""
    need = {}   # cell -> list of (ci, weight)
    for ci, ox in enumerate(oxs):
        a = int(a_vals[ox])
        need.setdefault(a, []).append((ci, 1.0 - wx_vals[ox]))
        need.setdefault(a + 1, []).append((ci, wx_vals[ox]))
    cells = sorted(need)
    wins = []
    i = 0
    while i < len(cells):
        xw = cells[i]
        cov = [(xw, 0)]
        if i + 1 < len(cells) and cells[i + 1] == xw + 1:
            cov.append((xw + 1, 1))
            i += 2
        else:
            i += 1
        entries = []
        for cell, q in cov:
            for (ci, wgt) in need[cell]:
                entries.append((ci, q, wgt))
        wins.append((xw, entries))
    return wins


def _prep_core(boxes, Him, Wim):
    """Tile metadata for one core's boxes."""
    B = boxes.shape[0]
    x0, wx, y0, y1, wy = _box_geometry(boxes, Him, Wim)

    box_rows = []
    for b in range(B):
        acc = {}
        for oy in range(7):
            w0 = 1.0 - wy[b, oy]
            w1 = wy[b, oy]
            if w0 > 0:
                acc.setdefault(y0[b, oy], np.zeros(7))[oy] += w0
            if w1 > 0:
                acc.setdefault(y1[b, oy], np.zeros(7))[oy] += w1
        ys = sorted(acc.keys())
        box_rows.append((ys, np.stack([acc[y] for y in ys], 0)))

    # pack boxes into tiles, splitting a box's ox-range when it doesn't fit
    tiles = []        # list of list of (b, oxs, wins)
    cur, cc, cd = [], 0, 0
    for b in range(B):
        ny = len(box_rows[b][0])
        oxs = list(range(7))
        while oxs:
            # how many cols fit by the column cap?
            room_c = MAX_COLS - cc
            if room_c == 0:
                tiles.append(cur)
                cur, cc, cd = [], 0, 0
                room_c = MAX_COLS
            take = oxs[:room_c]
            wins = _cover(x0[b], wx[b], take)
            while len(take) > 1 and cd + ny * len(wins) > MAX_DESC:
                take = take[:-1]
                wins = _cover(x0[b], wx[b], take)
            if cd + ny * len(wins) > MAX_DESC:
                tiles.append(cur)
                cur, cc, cd = [], 0, 0
                continue
            cur.append((b, take, wins))
            cc += len(take)
            cd += ny * len(wins)
            oxs = oxs[len(take):]
    if cur:
        tiles.append(cur)

    NT = len(tiles)
    idx = np.zeros((NT, MAX_DESC), np.int32)
    nval = np.zeros(NT, np.int32)
    cmq = np.zeros((NT, 2, MAX_DESC, MAX_COLS), np.float16)
    wyw = np.zeros((NT, MAX_DESC, OH), np.float16)
    meta = []
    for t, parts in enumerate(tiles):
        k = 0
        cl = 0
        colmap = []
        for (b, take, wins) in parts:
            ys, wrow = box_rows[b]
            ny = len(ys)
            for (xw, entries) in wins:
                for j, y in enumerate(ys):
                    idx[t, k + j] = y * W + xw
                    wyw[t, k + j] = wrow[j].astype(np.float16)
                    for (ci, q, wgt) in entries:
                        cmq[t, q, k + j, cl + ci] = np.float16(wgt)
                k += ny
            for ox in take:
                colmap.append((b, ox))
            cl += len(take)
        nval[t] = k
        meta.append(colmap)
    return idx, nval, cmq, wyw, meta


def _pack_core(idx, nval, cmq, wyw, NT):
    """Pad to NT tiles, order tiles (emptiest last per call), build device
    tensors. Returns (idxw, cmq_dev, wyw_dev, counts, order)."""
    nt = idx.shape[0]
    NCALLS = (NT + CALL_TILES - 1) // CALL_TILES
    # order tiles: sort by valid count desc (pad tiles = emptiest, at the
    # back); each call takes CALL_TILES-1 from the front and its trailing
    # slot from the back, so the trailing (neg-padded) tile skips the most
    # gather bytes.
    assert NT - nt <= NCALLS, "more pad tiles than calls"
    ext = sorted(range(nt), key=lambda t: -nval[t]) + [-1] * (NT - nt)
    front, back = 0, NT - 1
    order = []
    for c in range(NCALLS):
        want = min(CALL_TILES, NT - c * CALL_TILES)
        for _ in range(want - 1):
            order.append(ext[front])
            front += 1
        order.append(ext[back])
        back -= 1
    assert len(order) == NT and front == back + 1
    real = [t for t in order if t >= 0]
    assert sorted(real) == list(range(nt))

    flat = np.full(NT * MAX_DESC, -1 if NEG_PADS else 0, np.int16)
    cmq_p = np.zeros((NT, 2, MAX_DESC, MAX_COLS), np.float16)
    wyw_p = np.zeros((NT, MAX_DESC, OH), np.float16)
    counts = np.zeros((1, NCALLS), np.int32)
    for pos, t in enumerate(order):
        call = pos // CALL_TILES
        # trailing tile of a call may use skipped -1 pads -- but the first
        # 3 calls (first use of each gather pool buffer) must write every
        # slot so later skipped slots only ever expose stale finite data
        last_in_call = ((pos % CALL_TILES == CALL_TILES - 1)
                        or pos == NT - 1) and call >= GPOOL_BUFS
        base = pos * MAX_DESC
        nv = 0
        if t >= 0:
            nv = int(nval[t])
            flat[base:base + nv] = idx[t, :nv].astype(np.int16)
            cmq_p[pos] = cmq[t]
            wyw_p[pos] = wyw[t]
        if not (NEG_PADS and last_in_call):
            # interior tiles: pads must be valid indices (transferred)
            flat[base + nv:base + MAX_DESC] = 0
            counts[0, call] += MAX_DESC
        else:
            counts[0, call] += nv
    if not NEG_PADS:
        for c in range(NCALLS):
            w = min(CALL_TILES, NT - c * CALL_TILES)
            counts[0, c] = w * MAX_DESC
    # any call whose tiles are all pads still needs >= 1 valid descriptor
    for c in range(NCALLS):
        if counts[0, c] == 0:
            flat[c * CALL_TILES * MAX_DESC] = 0
            counts[0, c] = 1

    idxw = np.zeros((16, NT * 8), np.int16)
    j = np.arange(NT * MAX_DESC)
    idxw[j % 16, j // 16] = flat
    idxw = np.tile(idxw, (8, 1))

    cmq_dev = np.ascontiguousarray(
        cmq_p.transpose(2, 0, 1, 3)).reshape(128, NT * 2 * MAX_COLS)
    wyw_dev = np.ascontiguousarray(
        wyw_p.transpose(1, 0, 2)).reshape(128, NT * OH)
    return idxw, cmq_dev, wyw_dev, counts, order


_CACHE = {}


def _get_compiled(NT):
    key = ("nc", NT)
    if key in _CACHE:
        return _CACHE[key]
    import concourse.bacc as bacc
    import concourse.tile as tile
    nc = bacc.Bacc("TRN2", target_bir_lowering=False, debug=False,
                   num_swdge_queues=4)
    with tile.TileContext(nc) as tc:
        _build(nc, tc, NT)
    nc.compile()
    _CACHE[key] = nc
    return nc


def _run(feats, boxes, Him, Wim, trace=False, tmpdir=None):
    from concourse.bass_utils import run_bass_kernel_spmd

    table = np.zeros((HW + 1, C), np.float16)
    table[:HW] = feats.transpose(1, 2, 0).reshape(HW, C).astype(np.float16)

    preps = []
    for i in range(N_CORES):
        preps.append(_prep_core(boxes[i * B_LOCAL:(i + 1) * B_LOCAL],
                                Him, Wim))
    NT = max(p[0].shape[0] for p in preps)
    nc = _get_compiled(NT)

    in_maps = []
    orders = []
    for i in range(N_CORES):
        idx, nval, cmq, wyw, _ = preps[i]
        idxw, cmq_dev, wyw_dev, counts, order = _pack_core(
            idx, nval, cmq, wyw, NT)
        orders.append(order)
        in_maps.append({"feats_t": table, "idxw": idxw, "cmq": cmq_dev,
                        "wyw": wyw_dev, "cnt": counts})
    res = run_bass_kernel_spmd(nc, in_maps, list(range(N_CORES)),
                               trace=trace, tmpdir=tmpdir)

    out = np.zeros((B_TOTAL, C, OH, OW), np.float32)
    for i in range(N_CORES):
        o = np.asarray(res.results[i]["out3"], np.float32)
        o = o.reshape(128, NT, C)
        meta = preps[i][4]
        for pos, t in enumerate(orders[i]):
            if t < 0:
                continue
            colmap = meta[t]
            blk = o[:len(colmap) * OH, pos, :]      # [ncols*7, C]
            for ci, (b, ox) in enumerate(colmap):
                out[i * B_LOCAL + b, :, :, ox] = blk[ci * 7:(ci + 1) * 7].T
    return out, res


def kernel(**inputs):
    feats = np.asarray(inputs["feats"], dtype=np.float32)
    boxes = np.asarray(inputs["boxes"], dtype=np.float32)
    Him = int(inputs["image_height"])
    Wim = int(inputs["image_width"])
    out, _ = _run(feats, boxes, Him, Wim, trace=False)
    return out


# revision 24
# speedup vs baseline: 1.1177x; 1.0707x over previous
"""Bilinear RoI pooling (grid_sample style) on 8 Trainium2 NeuronCores.

Strategy (data-parallel over boxes; all coordinate math host-side):
  - The affine grid is axis-aligned, so sampling is separable: each output
    point (b, oy, ox) is a 2x2 bilinear blend. Per box only ~6 distinct
    feature rows are touched by all 7 oy grid rows (box heights are 8-64 px
    on a stride-8 map), so the kernel gathers one descriptor per
    (box, x-window, distinct-y-row) instead of one per sample point: a 2C
    fp16 window [f(y,x), f(y,x+1)] from a row-major [HW+1, C] table
    (elem_step=C, elem_size=2C). The x-windows are a minimal cover of the
    cells the box's 7 ox columns touch (narrow boxes share windows).
  - Descriptors are packed into tiles of <=128 descriptors covering <=18
    output columns (col = (b, ox); M = 7*18 = 126 output points <= 128).
  - Per tile, lhsT[k, (cl, oy)] factorizes as cmq[q][k, cl] * wyw[k, oy]
    (x-corner weight folded into the column mask), densified on-device by
    one DVE broadcast-multiply per (chunk, q). Two accumulating fp16
    matmuls per tile (q = left/right C half of the gathered window) produce
    PSUM [126 pts, C]; PSUM -> SBUF (alternating DVE/ACT) -> DRAM fp16.
  - Gathers use prepare_only SWDGE + trigger_dma so Pool-engine descriptor
    prep overlaps the DMA transfers. Each call's trailing pad descriptors
    are -1 (skipped by the DGE); the per-call valid count is read from an
    input tensor into a Pool register (per-core counts differ).
  - Host transposes the per-tile point blocks back to [B, C, 7, 7].
"""
import sys
import numpy as np

sys.path.insert(0, "/opt/trn_rl_repo")

OH = OW = 7
C, H, W = 512, 64, 256
HW = H * W
B_TOTAL = 2048
N_CORES = 8
B_LOCAL = B_TOTAL // N_CORES
MAX_COLS = 18            # columns per tile -> M = 126
MAX_DESC = 128           # descriptors per tile (K partitions)
M = MAX_COLS * OH        # 126
CALL_TILES = 8           # tiles per dma_gather call
GPOOL_BUFS = 6           # gather pool depth (pipeline)
PREPARE_ONLY = False      # SWDGE prepare_only + trigger_dma pipelining
NEG_PADS = True          # trailing -1 pads + per-call register counts


def _call_sizes(NT):
    """Call sizes ramp down at the end so the final transfers drain fast,
    and start small so the first matmuls begin early."""
    start = [2, 4]
    end = [4, 2]
    body = NT - sum(start) - sum(end)
    assert body > 0
    sizes = start + [CALL_TILES] * (body // CALL_TILES)
    if body % CALL_TILES:
        sizes.append(body % CALL_TILES)
    sizes += end
    assert sum(sizes) == NT
    return sizes


def _build(nc, tc, NT):
    from contextlib import ExitStack
    import concourse.mybir as mybir
    from concourse import bass

    f32 = mybir.dt.float32
    f16 = mybir.dt.float16
    i16 = mybir.dt.int16
    i32 = mybir.dt.int32
    A = mybir.AluOpType
    Pool = mybir.EngineType.Pool

    sizes = _call_sizes(NT)
    NCALLS = len(sizes)

    feats_t = nc.dram_tensor("feats_t", [HW + 1, C], f16, kind="ExternalInput")
    idxw_d = nc.dram_tensor("idxw", [128, NT * 8], i16, kind="ExternalInput")
    cmq_d = nc.dram_tensor("cmq", [128, NT * 2 * MAX_COLS], f16,
                           kind="ExternalInput")
    wyw_d = nc.dram_tensor("wyw", [128, NT * OH], f16, kind="ExternalInput")
    cnt_d = nc.dram_tensor("cnt", [1, NCALLS], i32, kind="ExternalInput")
    out_d = nc.dram_tensor("out3", [128, NT * C], f16, kind="ExternalOutput")

    es = ExitStack()
    idx0_s = es.enter_context(
        nc.sbuf_tensor("idx0_s", [128, sizes[0] * 8], i16))
    idx_s = es.enter_context(nc.sbuf_tensor("idx_s", [128, NT * 8], i16))
    cmq_s = es.enter_context(
        nc.sbuf_tensor("cmq_s", [128, NT, 2, MAX_COLS], f16))
    wyw_s = es.enter_context(nc.sbuf_tensor("wyw_s", [128, NT, OH], f16))
    cnt_s = es.enter_context(nc.sbuf_tensor("cnt_s", [1, NCALLS], i32))

    # gather source: row pitch C, window 2C (rows i and i+1 = x, x+1)
    src_ap = bass.AP(feats_t, 0, [[C, HW], [1, 2 * C]])

    ncmq = NT * 2 * MAX_COLS
    nwyw = NT * OH

    dma_sems = [nc.alloc_semaphore(f"swdge_dma{q}") for q in range(4)]

    with tc.tile_pool(name="gpool", bufs=GPOOL_BUFS) as gpool, \
         tc.tile_pool(name="wpool", bufs=3) as wpool, \
         tc.tile_pool(name="spool", bufs=6) as spool, \
         tc.tile_pool(name="psum", bufs=8, space="PSUM") as psum_pool:
        # split the idx/cnt loads so the first gather can start as soon as
        # its own slice lands, instead of after all weight tensors
        c0 = sizes[0] * 8
        nc.sync.dma_start(out=idx0_s[:, :], in_=idxw_d[:, :c0])
        nc.sync.dma_start(out=cnt_s[:, :], in_=cnt_d[:, :])
        nc.sync.dma_start(out=idx_s[:, c0:], in_=idxw_d[:, c0:])
        nc.sync.dma_start(
            out=bass.AP(cmq_s, 0, [[ncmq, 128], [1, ncmq]]), in_=cmq_d[:, :])
        nc.sync.dma_start(
            out=bass.AP(wyw_s, 0, [[nwyw, 128], [1, nwyw]]), in_=wyw_d[:, :])

        t0 = 0
        for ci in range(NCALLS):
            k = sizes[ci]
            q = ci % 4
            Gt = gpool.tile([128, CALL_TILES, 2 * C], f16, name="Gt")
            if NEG_PADS and ci >= GPOOL_BUFS:
                reg = nc.alloc_registers(f"cnt{ci}", engines=[Pool])
                nc.regs_load(reg, cnt_s[0:1, ci:ci + 1])
                cnt_val = nc.snap(reg, donate=True, min_val=1,
                                  max_val=k * MAX_DESC)
            else:
                # first GPOOL_BUFS calls are packed all-valid
                cnt_val = k * 128
            iap = (idx0_s[:, :k * 8] if ci == 0
                   else idx_s[:, t0 * 8: (t0 + k) * 8])
            nc.gpsimd.dma_gather(
                out_ap=Gt[:, :k, :], in_ap=src_ap,
                idxs_ap=iap,
                num_idxs=k * 128, num_idxs_reg=cnt_val,
                elem_size=2 * C, elem_step=C, queue_num=q)
            # densify lhsT: wden[p, ti, q, cl*7+oy] = cmq[p,t,q,cl]*wyw[p,t,oy]
            wden = wpool.tile([128, CALL_TILES, 2, M], f16, name="wden")
            for xq in range(2):
                nc.vector.tensor_tensor(
                    out=wden[:, :k, xq, :],
                    in0=bass.AP(cmq_s, (t0 * 2 + xq) * MAX_COLS,
                                [[NT * 2 * MAX_COLS, 128],
                                 [2 * MAX_COLS, k], [1, MAX_COLS], [0, OH]]),
                    in1=bass.AP(wyw_s, t0 * OH,
                                [[NT * OH, 128], [OH, k], [0, MAX_COLS],
                                 [1, OH]]),
                    op=A.mult)
            stage = spool.tile([128, CALL_TILES, C], f16, name="stage")
            for ti in range(k):
                t = t0 + ti
                ps = psum_pool.tile([128, C], f32, name="ps")
                for xq in range(2):
                    nc.tensor.matmul(
                        out=ps[:M, :],
                        lhsT=wden[:, ti, xq, :],
                        rhs=Gt[:, ti, xq * C:(xq + 1) * C],
                        start=(xq == 0), stop=(xq == 1))
                dst = stage[:M, ti, :]
                if t % 2 == 0:
                    nc.vector.tensor_copy(out=dst, in_=ps[:M, :])
                else:
                    nc.scalar.activation(
                        out=dst, in_=ps[:M, :],
                        func=mybir.ActivationFunctionType.Copy)
            nc.sync.dma_start(
                out=bass.AP(out_d, t0 * C, [[NT * C, M], [1, k * C]]),
                in_=stage[:M, :k, :])
            t0 += k


def _box_geometry(boxes, Him, Wim):
    xc, yc, bw, bh = [boxes[:, i].astype(np.float64) for i in range(4)]
    gl = np.linspace(-1.0, 1.0, 7)
    gx = gl[None, :] * ((bw - 1) / (Wim - 1))[:, None] \
        + ((2 * xc - Wim - 1) / (Wim - 1))[:, None]
    gy = gl[None, :] * ((bh - 1) / (Him - 1))[:, None] \
        + ((2 * yc - Him - 1) / (Him - 1))[:, None]
    ix = np.clip((gx + 1) * 0.5 * (W - 1), 0.0, W - 1.0)
    iy = np.clip((gy + 1) * 0.5 * (H - 1), 0.0, H - 1.0)
    x0 = np.floor(ix).astype(np.int64)
    y0 = np.floor(iy).astype(np.int64)
    wx = ix - x0
    wy = iy - y0
    y1 = np.minimum(y0 + 1, H - 1)
    return x0, wx, y0, y1, wy


def _cover(a_vals, wx_vals, oxs):
    """Minimal 2-cell window cover of the cells needed by columns `oxs`.

    Returns list of (xw, [(col_local_idx, q, weight), ...]) where q selects
    the left/right C half of the gathered window."""
    need = {}   # cell -> list of (ci, weight)
    for ci, ox in enumerate(oxs):
        a = int(a_vals[ox])
        need.setdefault(a, []).append((ci, 1.0 - wx_vals[ox]))
        need.setdefault(a + 1, []).append((ci, wx_vals[ox]))
    cells = sorted(need)
    wins = []
    i = 0
    while i < len(cells):
        xw = cells[i]
        cov = [(xw, 0)]
        if i + 1 < len(cells) and cells[i + 1] == xw + 1:
            cov.append((xw + 1, 1))
            i += 2
        else:
            i += 1
        entries = []
        for cell, q in cov:
            for (ci, wgt) in need[cell]:
                entries.append((ci, q, wgt))
        wins.append((xw, entries))
    return wins


def _prep_core(boxes, Him, Wim):
    """Tile metadata for one core's boxes."""
    B = boxes.shape[0]
    x0, wx, y0, y1, wy = _box_geometry(boxes, Him, Wim)

    box_rows = []
    for b in range(B):
        acc = {}
        for oy in range(7):
            w0 = 1.0 - wy[b, oy]
            w1 = wy[b, oy]
            if w0 > 0:
                acc.setdefault(y0[b, oy], np.zeros(7))[oy] += w0
            if w1 > 0:
                acc.setdefault(y1[b, oy], np.zeros(7))[oy] += w1
        ys = sorted(acc.keys())
        box_rows.append((ys, np.stack([acc[y] for y in ys], 0)))

    # pack boxes into tiles, splitting a box's ox-range when it doesn't fit
    tiles = []        # list of list of (b, oxs, wins)
    cur, cc, cd = [], 0, 0
    for b in range(B):
        ny = len(box_rows[b][0])
        oxs = list(range(7))
        while oxs:
            # how many cols fit by the column cap?
            room_c = MAX_COLS - cc
            if room_c == 0:
                tiles.append(cur)
                cur, cc, cd = [], 0, 0
                room_c = MAX_COLS
            take = oxs[:room_c]
            wins = _cover(x0[b], wx[b], take)
            while len(take) > 1 and cd + ny * len(wins) > MAX_DESC:
                take = take[:-1]
                wins = _cover(x0[b], wx[b], take)
            if cd + ny * len(wins) > MAX_DESC:
                tiles.append(cur)
                cur, cc, cd = [], 0, 0
                continue
            cur.append((b, take, wins))
            cc += len(take)
            cd += ny * len(wins)
            oxs = oxs[len(take):]
    if cur:
        tiles.append(cur)

    NT = len(tiles)
    idx = np.zeros((NT, MAX_DESC), np.int32)
    nval = np.zeros(NT, np.int32)
    cmq = np.zeros((NT, 2, MAX_DESC, MAX_COLS), np.float16)
    wyw = np.zeros((NT, MAX_DESC, OH), np.float16)
    meta = []
    for t, parts in enumerate(tiles):
        k = 0
        cl = 0
        colmap = []
        for (b, take, wins) in parts:
            ys, wrow = box_rows[b]
            ny = len(ys)
            for (xw, entries) in wins:
                for j, y in enumerate(ys):
                    idx[t, k + j] = y * W + xw
                    wyw[t, k + j] = wrow[j].astype(np.float16)
                    for (ci, q, wgt) in entries:
                        cmq[t, q, k + j, cl + ci] = np.float16(wgt)
                k += ny
            for ox in take:
                colmap.append((b, ox))
            cl += len(take)
        nval[t] = k
        meta.append(colmap)
    return idx, nval, cmq, wyw, meta


def _pack_core(idx, nval, cmq, wyw, NT):
    """Pad to NT tiles, order tiles (emptiest last per call), build device
    tensors. Returns (idxw, cmq_dev, wyw_dev, counts, order)."""
    nt = idx.shape[0]
    sizes = _call_sizes(NT)
    NCALLS = len(sizes)
    # order tiles: sort by valid count desc (pad tiles = emptiest, at the
    # back); each call takes its first slots from the front and its trailing
    # slot from the back, so the trailing (neg-padded) tile skips the most
    # gather bytes.
    assert NT - nt <= NCALLS, "more pad tiles than calls"
    ext = sorted(range(nt), key=lambda t: -nval[t]) + [-1] * (NT - nt)
    front, back = 0, NT - 1
    order = []
    callof = []
    for c, want in enumerate(sizes):
        # early (all-valid) calls take only the fullest tiles; later calls
        # put an emptiest tile in the trailing slot to maximize skipped pads
        nf = want if c < GPOOL_BUFS else want - 1
        for _ in range(nf):
            order.append(ext[front])
            front += 1
        if nf < want:
            order.append(ext[back])
            back -= 1
        callof += [c] * want
    assert len(order) == NT and front == back + 1
    real = [t for t in order if t >= 0]
    assert sorted(real) == list(range(nt))
    call_end = np.cumsum(sizes) - 1   # last position of each call

    flat = np.full(NT * MAX_DESC, -1 if NEG_PADS else 0, np.int16)
    cmq_p = np.zeros((NT, 2, MAX_DESC, MAX_COLS), np.float16)
    wyw_p = np.zeros((NT, MAX_DESC, OH), np.float16)
    counts = np.zeros((1, NCALLS), np.int32)
    for pos, t in enumerate(order):
        call = callof[pos]
        # trailing tile of a call may use skipped -1 pads -- but the first
        # GPOOL_BUFS calls (first use of each gather pool buffer) must write
        # every slot so later skipped slots only expose stale finite data
        last_in_call = pos == call_end[call] and call >= GPOOL_BUFS
        base = pos * MAX_DESC
        nv = 0
        if t >= 0:
            nv = int(nval[t])
            flat[base:base + nv] = idx[t, :nv].astype(np.int16)
            cmq_p[pos] = cmq[t]
            wyw_p[pos] = wyw[t]
        if not (NEG_PADS and last_in_call):
            # interior tiles: pads must be valid indices (transferred)
            flat[base + nv:base + MAX_DESC] = 0
            counts[0, call] += MAX_DESC
        else:
            counts[0, call] += nv
    if not NEG_PADS:
        for c, w in enumerate(sizes):
            counts[0, c] = w * MAX_DESC
    # any call whose tiles are all pads still needs >= 1 valid descriptor
    call_start = np.concatenate([[0], np.cumsum(sizes)[:-1]])
    for c in range(NCALLS):
        if counts[0, c] == 0:
            flat[call_start[c] * MAX_DESC] = 0
            counts[0, c] = 1

    idxw = np.zeros((16, NT * 8), np.int16)
    j = np.arange(NT * MAX_DESC)
    idxw[j % 16, j // 16] = flat
    idxw = np.tile(idxw, (8, 1))

    cmq_dev = np.ascontiguousarray(
        cmq_p.transpose(2, 0, 1, 3)).reshape(128, NT * 2 * MAX_COLS)
    wyw_dev = np.ascontiguousarray(
        wyw_p.transpose(1, 0, 2)).reshape(128, NT * OH)
    return idxw, cmq_dev, wyw_dev, counts, order


_CACHE = {}


def _get_compiled(NT):
    key = ("nc", NT)
    if key in _CACHE:
        return _CACHE[key]
    import concourse.bacc as bacc
    import concourse.tile as tile
    nc = bacc.Bacc("TRN2", target_bir_lowering=False, debug=False,
                   num_swdge_queues=4)
    with tile.TileContext(nc) as tc:
        _build(nc, tc, NT)
    nc.compile()
    _CACHE[key] = nc
    return nc


def _run(feats, boxes, Him, Wim, trace=False, tmpdir=None):
    from concourse.bass_utils import run_bass_kernel_spmd

    table = np.zeros((HW + 1, C), np.float16)
    table[:HW] = feats.transpose(1, 2, 0).reshape(HW, C).astype(np.float16)

    preps = []
    for i in range(N_CORES):
        preps.append(_prep_core(boxes[i * B_LOCAL:(i + 1) * B_LOCAL],
                                Him, Wim))
    NT = max(p[0].shape[0] for p in preps)
    nc = _get_compiled(NT)

    in_maps = []
    orders = []
    for i in range(N_CORES):
        idx, nval, cmq, wyw, _ = preps[i]
        idxw, cmq_dev, wyw_dev, counts, order = _pack_core(
            idx, nval, cmq, wyw, NT)
        orders.append(order)
        in_maps.append({"feats_t": table, "idxw": idxw, "cmq": cmq_dev,
                        "wyw": wyw_dev, "cnt": counts})
    res = run_bass_kernel_spmd(nc, in_maps, list(range(N_CORES)),
                               trace=trace, tmpdir=tmpdir)

    out = np.zeros((B_TOTAL, C, OH, OW), np.float32)
    for i in range(N_CORES):
        o = np.asarray(res.results[i]["out3"], np.float32)
        o = o.reshape(128, NT, C)
        meta = preps[i][4]
        for pos, t in enumerate(orders[i]):
            if t < 0:
                continue
            colmap = meta[t]
            blk = o[:len(colmap) * OH, pos, :]      # [ncols*7, C]
            for ci, (b, ox) in enumerate(colmap):
                out[i * B_LOCAL + b, :, :, ox] = blk[ci * 7:(ci + 1) * 7].T
    return out, res


def kernel(**inputs):
    feats = np.asarray(inputs["feats"], dtype=np.float32)
    boxes = np.asarray(inputs["boxes"], dtype=np.float32)
    Him = int(inputs["image_height"])
    Wim = int(inputs["image_width"])
    out, _ = _run(feats, boxes, Him, Wim, trace=False)
    return out
